# revision 14
# baseline (speedup 1.0000x reference)
"""Trainium2 Bass kernel for nn_PartialConvLayer (partial conv 3x3 + mask
update + BatchNorm(batch stats) + ReLU), data-parallel over batch on 8 cores.

v2: bf16 conv with 128-deep contraction (tap pairing via column-shifted xm
copy), pre-BN activations kept in SBUF (bf16) instead of a DRAM bounce,
row-pair mask-sum matmuls, SBUF->SBUF DMA relayouts only.

Math (per image):
  update = conv(mask, ones(Cin,3,3)), pad 1          # integer in {0..576}
  u      = clip(update, 0, 1)
  mr     = 576 / (update + 1e-6) * u
  conv   = conv(x*mask, W), pad 1                    # no bias
  out    = conv * mr + b * u = (conv + (b/576) (x) v) * mr,  v = u*(update+eps)
  BN over (N,H,W) batch stats (all-reduced across cores), then ReLU.
Returns (out, broadcast(update_clipped)).
"""
import os
import numpy as np
import ml_dtypes
from contextlib import ExitStack

import concourse.bass as bass
import concourse.tile as tile
from concourse import mybir, bacc
from concourse import library_config
from concourse.bass_utils import run_bass_kernel_spmd

F32 = mybir.dt.float32
F32R = mybir.dt.float32r
BF16 = mybir.dt.bfloat16
ALU = mybir.AluOpType
ACTF = mybir.ActivationFunctionType

CIN = 64
COUT = 128
W_ = 256
KS = 3
EPS_MASK = 1e-6
EPS_BN = 1e-5
SLIDE = float(CIN * KS * KS)   # 576


def build_nc(n_cores=8, H=256, B=8):
    """SPMD program for one core holding one [CIN, H, W_] image."""
    nblk = H // B                    # row blocks per core
    NR = B + 2                       # rows per block incl halo
    nchunk = (H * W_) // 512         # 2-row conv chunks per core
    TOT = float(n_cores * H * W_)    # BN count

    nc = bacc.Bacc(None, num_devices=n_cores)

    X = nc.dram_tensor("x", [CIN, H * W_], F32, kind="ExternalInput")
    M = nc.dram_tensor("mask", [CIN, H * W_], F32R, kind="ExternalInput")
    WTB = nc.dram_tensor("wtb", [128, 6 * COUT], BF16, kind="ExternalInput")
    BPB = nc.dram_tensor("bpb", [1, COUT], BF16, kind="ExternalInput")
    ONES2 = nc.dram_tensor("ones2", [128, 2], F32R, kind="ExternalInput")
    T3V = nc.dram_tensor("t3v", [NR, B], F32R, kind="ExternalInput")
    GAM = nc.dram_tensor("gam", [COUT, 1], F32, kind="ExternalInput")
    BET = nc.dram_tensor("bet", [COUT, 1], F32, kind="ExternalInput")

    OUT = nc.dram_tensor("out", [COUT, H * W_], F32, kind="ExternalOutput")
    UPD = nc.dram_tensor("upd", [H, W_], F32, kind="ExternalOutput")

    cc_in = nc.dram_tensor("ccin", [COUT, 2], F32)
    cc_out = nc.dram_tensor("ccout", [COUT, 2], F32,
                            addr_space="Shared" if n_cores > 4 else "Local")

    with tile.TileContext(nc) as tc, ExitStack() as ctx:
        nc.gpsimd.load_library(library_config.mlp)

        const = ctx.enter_context(tc.tile_pool(name="const", bufs=1))
        iox = ctx.enter_context(tc.tile_pool(name="iox", bufs=1))
        iom = ctx.enter_context(tc.tile_pool(name="iom", bufs=1))
        xmp = ctx.enter_context(tc.tile_pool(name="xmp", bufs=1))
        sev = ctx.enter_context(tc.tile_pool(name="sev", bufs=1))
        usb = ctx.enter_context(tc.tile_pool(name="usb", bufs=2))
        upd1 = ctx.enter_context(tc.tile_pool(name="upd1", bufs=1))
        upd2 = ctx.enter_context(tc.tile_pool(name="upd2", bufs=2))
        bcp = ctx.enter_context(tc.tile_pool(name="bcp", bufs=2))
        sqp = ctx.enter_context(tc.tile_pool(name="sqp", bufs=1))
        otp = ctx.enter_context(tc.tile_pool(name="otp", bufs=2))
        psc = ctx.enter_context(tc.tile_pool(name="psc", bufs=5, space="PSUM"))
        pss = ctx.enter_context(tc.tile_pool(name="pss", bufs=2, space="PSUM"))
        psv = ctx.enter_context(tc.tile_pool(name="psv", bufs=1, space="PSUM"))

        # ---- constants ----
        wt_b = const.tile([128, 6 * COUT], BF16)
        nc.sync.dma_start(wt_b[:], WTB[:])
        bp_b = const.tile([1, COUT], BF16)
        nc.sync.dma_start(bp_b[:], BPB[:])
        ones2_t = const.tile([128, 2], F32R)
        nc.sync.dma_start(ones2_t[:], ONES2[:])
        t3v_t = const.tile([NR, B], F32R)
        nc.sync.dma_start(t3v_t[:], T3V[:])
        gam_t = const.tile([COUT, 1], F32)
        nc.sync.dma_start(gam_t[:], GAM[:])
        bet_t = const.tile([COUT, 1], F32)
        nc.sync.dma_start(bet_t[:], BET[:])
        eps_t = const.tile([COUT, 1], F32)
        nc.vector.memset(eps_t[:], EPS_BN)
        sum_slots = const.tile([COUT, nchunk], F32)
        sq_slots = const.tile([COUT, nchunk // 2], F32)
        prebn = const.tile([COUT, H * W_], BF16)

        # two persistent padded xm buffers; guard cols zeroed once.
        # T0 (parts 0:64): col c = img col c-1 (writes 1:257); col 0 guard.
        # T1 (parts 64:128): col c = img col c (writes 0:256); col 256 guard.
        xm_tiles = []
        for i in range(2):
            t = const.tile([128, NR * 258], BF16, tag=f"xm{i}")
            nc.vector.memset(t[:, :].bitcast(F32), 0.0)
            xm_tiles.append(t)
        # s_rows tiles with zero guard cols 0 and 257
        sr_tiles = []
        for i in range(2):
            t = const.tile([NR, 258], F32R, tag=f"sr{i}")
            nc.vector.memset(t[:, :].bitcast(F32), 0.0)
            sr_tiles.append(t)
        # v (row 0) / mru (row 1) strips per block, bf16
        vs_tiles = []
        ms_tiles = []
        for i in range(2):
            t = const.tile([1, B * W_], BF16, tag=f"vs{i}")
            vs_tiles.append(t)
            t2 = const.tile([1, B * W_], BF16, tag=f"ms{i}")
            ms_tiles.append(t2)

        # ---- software-pipelined block loop: iteration k runs the mask/
        # update pipeline for block k+1 interleaved with convs of block k.
        blk_state = {}

        def mask_load(k):
            r0 = k * B
            first, last = (k == 0), (k == nblk - 1)
            x_t = iox.tile([CIN, NR * W_], F32, tag="x_t")
            m_t = iom.tile([128, NR * W_], F32R, tag="m_t")
            lo = max(r0 - 1, 0)
            hi = min(r0 + B + 1, H)
            dst0 = (lo - (r0 - 1)) * W_
            nr = hi - lo
            nc.sync.dma_start(
                x_t[:, dst0:dst0 + nr * W_],
                bass.AP(tensor=X, offset=lo * W_, ap=[[H * W_, CIN], [1, nr * W_]]))
            nc.sync.dma_start(
                m_t[0:CIN, dst0:dst0 + nr * W_],
                bass.AP(tensor=M, offset=lo * W_, ap=[[H * W_, CIN], [1, nr * W_]]))
            if first:
                nc.vector.memset(x_t[:, 0:W_], 0.0)
                nc.vector.memset(m_t[0:CIN, 0:W_].bitcast(F32), 0.0)
            if last:
                nc.vector.memset(x_t[:, (NR - 1) * W_:], 0.0)
                nc.vector.memset(m_t[0:CIN, (NR - 1) * W_:].bitcast(F32), 0.0)
            # mask rows shifted +1 into upper partitions (SBUF->SBUF DMA)
            nc.sync.dma_start(m_t[64:128, 0:(NR - 1) * W_],
                              m_t[0:CIN, W_:NR * W_])
            blk_state[k] = (x_t, m_t)

        def mask_strips(k):
            x_t, m_t = blk_state[k]
            m3 = m_t[:, :].rearrange("p (r c) -> p r c", c=W_)
            # strip q: rhs rows {4q, 4q+2}; out [2,512] =
            #   [s(4q), s(4q+2); s(4q+1), s(4q+3)]   (row indices in block)
            s_evac = sev.tile([2, NR * W_ // 2], F32R, tag="s_evac")
            nq = NR // 4
            for q in range(nq):
                ps_s = pss.tile([2, 512], F32, tag="ps_s")
                nc.tensor.matmul(ps_s[:], ones2_t[:],
                                 m3[:, 4 * q:4 * q + 4:2, :],
                                 start=True, stop=True)
                nc.scalar.copy(s_evac[:, q * 512:(q + 1) * 512], ps_s[:])
            ps_s = pss.tile([2, 512], F32, tag="ps_s")
            nc.tensor.matmul(ps_s[:, 0:256], ones2_t[:], m3[:, NR - 2, :],
                             start=True, stop=True)
            nc.scalar.copy(s_evac[:, nq * 512:nq * 512 + 256], ps_s[:, 0:256])
            # relayout: s_evac row h holds rows (4q+2w+h) in order
            s_rows = sr_tiles[k % 2]
            nc.sync.dma_start(s_rows[0:NR - 1:2, 1:257], s_evac[0:1, :])
            nc.sync.dma_start(s_rows[1:NR:2, 1:257], s_evac[1:2, :])

        def xm_build(k):
            x_t, m_t = blk_state[k]
            m3 = m_t[:, :].rearrange("p (r c) -> p r c", c=W_)
            xm = xm_tiles[k % 2]
            xm3 = xm[:, :].rearrange("p (r c) -> p r c", c=258)
            x3 = x_t[:, :].rearrange("p (r c) -> p r c", c=W_)
            nc.vector.tensor_tensor(
                xm3[0:64, :, 1:257], x3[:, :, :], m3[0:CIN, :, :], op=ALU.mult)
            nc.vector.tensor_tensor(
                xm3[64:128, :, 0:256], x3[:, :, :], m3[0:CIN, :, :], op=ALU.mult)

        def mask_update(k):
            r0 = k * B
            s_rows = sr_tiles[k % 2]
            ps_V = psv.tile([B, 258], F32, tag="ps_V")
            nc.tensor.matmul(ps_V[:], t3v_t[:], s_rows[:, :], start=True,
                             stop=True)
            u_sb = usb.tile([B, 258], F32, tag="u_sb")
            nc.scalar.copy(u_sb[:], ps_V[:])

            vh = upd1.tile([B, W_], F32, tag="vh")
            nc.vector.tensor_add(vh[:], u_sb[:, 0:256], u_sb[:, 1:257])
            nc.vector.tensor_add(vh[:], vh[:], u_sb[:, 2:258])
            u_clip = upd2.tile([B, W_], F32, tag="u_clip")
            nc.vector.tensor_scalar_min(u_clip[:], vh[:], 1.0)
            upde = upd1.tile([B, W_], F32, tag="upde")
            nc.vector.tensor_scalar_add(upde[:], vh[:], EPS_MASK)
            rec = upd1.tile([B, W_], F32, tag="rec")
            nc.vector.reciprocal(rec[:], upde[:])
            mru_r = upd2.tile([B, W_], BF16, tag="mru_r")
            nc.vector.scalar_tensor_tensor(
                out=mru_r[:], in0=rec[:], scalar=SLIDE, in1=u_clip[:],
                op0=ALU.mult, op1=ALU.mult)
            v_r = upd2.tile([B, W_], BF16, tag="v_r")
            nc.vector.scalar_tensor_tensor(
                out=v_r[:], in0=upde[:], scalar=1.0, in1=u_clip[:],
                op0=ALU.mult, op1=ALU.mult)

            nc.sync.dma_start(
                bass.AP(tensor=UPD, offset=r0 * W_, ap=[[W_, B], [1, W_]]),
                u_clip[:])
            vs = vs_tiles[k % 2]
            ms = ms_tiles[k % 2]
            nc.sync.dma_start(vs[:], v_r[:])
            nc.sync.dma_start(ms[:], mru_r[:])

        def conv_chunks(k, js):
            r0 = k * B
            xm = xm_tiles[k % 2]
            xm3 = xm[:, :].rearrange("p (r c) -> p r c", c=258)
            vs = vs_tiles[k % 2]
            ms = ms_tiles[k % 2]
            for j in js:
                mru_bc = bcp.tile([128, 512], BF16, tag="mru_bc")
                nc.gpsimd.partition_broadcast(
                    mru_bc[:], ms[0:1, j * 512:(j + 1) * 512])

                ps_c = psc.tile([COUT, 512], F32, tag="ps_c")
                for ky in range(3):
                    nc.tensor.matmul(
                        ps_c[:],
                        wt_b[:, ky * COUT:(ky + 1) * COUT],
                        xm3[:, 2 * j + ky:2 * j + ky + 2, 0:256],
                        start=(ky == 0), stop=False)
                for ky in range(3):
                    nc.tensor.matmul(
                        ps_c[:],
                        wt_b[64:128, (3 + ky) * COUT:(4 + ky) * COUT],
                        xm3[64:128, 2 * j + ky:2 * j + ky + 2, 1:257],
                        start=False, stop=False)
                nc.tensor.matmul(ps_c[:], bp_b[:],
                                 vs[0:1, j * 512:(j + 1) * 512],
                                 start=False, stop=True)

                ci = 4 * k + j
                col = (r0 + 2 * j) * W_
                nc.vector.scalar_tensor_tensor(
                    out=prebn[:, col:col + 512], in0=ps_c[:], scalar=0.0,
                    in1=mru_bc[:], op0=ALU.add, op1=ALU.mult,
                    accum_out=sum_slots[:, ci:ci + 1])
            # paired Square over both chunks (1024 cols)
            col = (r0 + 2 * js[0]) * W_
            sq_scr = sqp.tile([COUT, 1024], BF16, tag="sq_scr")
            nc.scalar.activation(
                sq_scr[:], prebn[:, col:col + 1024], ACTF.Square,
                accum_out=sq_slots[:, 2 * k + js[0] // 2:2 * k + js[0] // 2 + 1])

        # prologue: full mask pipeline for block 0
        mask_load(0)
        mask_strips(0)
        mask_update(0)
        xm_build(0)
        for k in range(nblk):
            if k + 1 < nblk:
                mask_load(k + 1)
            conv_chunks(k, [0, 1])
            if k + 1 < nblk:
                mask_strips(k + 1)
                xm_build(k + 1)
            conv_chunks(k, [2, 3])
            if k + 1 < nblk:
                mask_update(k + 1)
            blk_state.pop(k, None)

        # ---- BN stats: reduce, all-reduce, affine coeffs ----
        cc_sb = const.tile([COUT, 2], F32)
        nc.vector.tensor_reduce(cc_sb[:, 0:1], sum_slots[:],
                                axis=mybir.AxisListType.X, op=ALU.add)
        nc.vector.tensor_reduce(cc_sb[:, 1:2], sq_slots[:],
                                axis=mybir.AxisListType.X, op=ALU.add)
        nc.sync.dma_start(cc_in[:], cc_sb[:])
        nc.gpsimd.collective_compute(
            "AllReduce", ALU.add,
            replica_groups=[list(range(n_cores))],
            ins=[cc_in.ap().opt()], outs=[cc_out.ap().opt()])
        st_sb = const.tile([COUT, 2], F32)
        nc.sync.dma_start(st_sb[:], cc_out[:])
        mean_t = const.tile([COUT, 1], F32)
        nc.vector.tensor_scalar_mul(mean_t[:], st_sb[:, 0:1], 1.0 / TOT)
        e2_t = const.tile([COUT, 1], F32)
        nc.vector.tensor_scalar_mul(e2_t[:], st_sb[:, 1:2], 1.0 / TOT)
        msq_t = const.tile([COUT, 1], F32)
        nc.vector.tensor_mul(msq_t[:], mean_t[:], mean_t[:])
        var_t = const.tile([COUT, 1], F32)
        nc.vector.tensor_sub(var_t[:], e2_t[:], msq_t[:])
        std_t = const.tile([COUT, 1], F32)
        nc.scalar.activation(std_t[:], var_t[:], ACTF.Sqrt, bias=eps_t[:])
        rstd_t = const.tile([COUT, 1], F32)
        nc.vector.reciprocal(rstd_t[:], std_t[:])
        scale_t = const.tile([COUT, 1], F32)
        nc.vector.tensor_mul(scale_t[:], gam_t[:], rstd_t[:])
        tmp_t = const.tile([COUT, 1], F32)
        nc.vector.tensor_mul(tmp_t[:], mean_t[:], scale_t[:])
        bias_t = const.tile([COUT, 1], F32)
        nc.vector.tensor_sub(bias_t[:], bet_t[:], tmp_t[:])

        # ---- pass 2: out = relu(scale*prebn + bias) ----
        P2 = 1024
        for i in range(0, H * W_, P2):
            o_t = otp.tile([COUT, P2], F32, tag="o_t")
            nc.scalar.activation(o_t[:], prebn[:, i:i + P2], ACTF.Relu,
                                 bias=bias_t[:], scale=scale_t[:])
            nc.sync.dma_start(OUT[:, i:i + P2], o_t[:])

    return nc


def make_host_inputs(x_i, mask_i, W, b, gamma, beta, B=8):
    """Per-core in_map for one image shard (host-side constant prep)."""
    NR = B + 2
    WTb = np.zeros((128, 6 * COUT), np.float32)
    for ky in range(KS):
        # fused pair (ky, kx=0) lower / (ky, kx=1) upper
        WTb[0:64, ky * COUT:(ky + 1) * COUT] = W[:, :, ky, 0].T
        WTb[64:128, ky * COUT:(ky + 1) * COUT] = W[:, :, ky, 1].T
        # single (ky, kx=2) upper
        WTb[64:128, (3 + ky) * COUT:(4 + ky) * COUT] = W[:, :, ky, 2].T
    ones2 = np.zeros((128, 2), np.float32)
    ones2[0:64, 0] = 1.0
    ones2[64:128, 1] = 1.0
    T3V = np.zeros((NR, B), np.float32)
    for j in range(B):
        T3V[j:j + 3, j] = 1.0
    return {
        "x": np.ascontiguousarray(x_i, dtype=np.float32).reshape(CIN, -1),
        "mask": np.ascontiguousarray(mask_i, dtype=np.float32).reshape(CIN, -1),
        "wtb": WTb.astype(ml_dtypes.bfloat16),
        "bpb": (b / SLIDE).reshape(1, COUT).astype(ml_dtypes.bfloat16),
        "ones2": ones2,
        "t3v": T3V,
        "gam": gamma.reshape(COUT, 1).astype(np.float32),
        "bet": beta.reshape(COUT, 1).astype(np.float32),
    }


_NC_CACHE = {}


def kernel(x, mask, W, b, gamma, beta):
    x = np.asarray(x)
    mask = np.asarray(mask)
    W = np.asarray(W)
    b = np.asarray(b)
    gamma = np.asarray(gamma)
    beta = np.asarray(beta)
    N, _, H, _ = x.shape
    n_cores = N
    key = (n_cores, H)
    if key not in _NC_CACHE:
        nc = build_nc(n_cores=n_cores, H=H)
        nc.finalize()
        _NC_CACHE[key] = nc
    nc = _NC_CACHE[key]

    in_maps = [make_host_inputs(x[i], mask[i], W, b, gamma, beta)
               for i in range(n_cores)]
    res = run_bass_kernel_spmd(nc, in_maps, core_ids=list(range(n_cores)),
                               trace=bool(os.environ.get("KERNEL_TRACE")))
    out = np.stack([res.results[i]["out"].reshape(COUT, H, W_)
                    for i in range(n_cores)])
    upd = np.stack([res.results[i]["upd"] for i in range(n_cores)])
    update_full = np.broadcast_to(upd[:, None, :, :], (N, COUT, H, W_))
    kernel.last_result = res
    return out, update_full


# revision 15
# speedup vs baseline: 1.1394x; 1.1394x over previous
"""Trainium2 Bass kernel for nn_PartialConvLayer (partial conv 3x3 + mask
update + BatchNorm(batch stats) + ReLU), data-parallel over batch on 8 cores.

v2: bf16 conv with 128-deep contraction (tap pairing via column-shifted xm
copy), pre-BN activations kept in SBUF (bf16) instead of a DRAM bounce,
row-pair mask-sum matmuls, SBUF->SBUF DMA relayouts only.

Math (per image):
  update = conv(mask, ones(Cin,3,3)), pad 1          # integer in {0..576}
  u      = clip(update, 0, 1)
  mr     = 576 / (update + 1e-6) * u
  conv   = conv(x*mask, W), pad 1                    # no bias
  out    = conv * mr + b * u = (conv + (b/576) (x) v) * mr,  v = u*(update+eps)
  BN over (N,H,W) batch stats (all-reduced across cores), then ReLU.
Returns (out, broadcast(update_clipped)).
"""
import os
import numpy as np
import ml_dtypes
from contextlib import ExitStack

import concourse.bass as bass
import concourse.tile as tile
from concourse import mybir, bacc
from concourse import library_config
from concourse.bass_utils import run_bass_kernel_spmd

F32 = mybir.dt.float32
F32R = mybir.dt.float32r
BF16 = mybir.dt.bfloat16
ALU = mybir.AluOpType
ACTF = mybir.ActivationFunctionType

CIN = 64
COUT = 128
W_ = 256
KS = 3
EPS_MASK = 1e-6
EPS_BN = 1e-5
SLIDE = float(CIN * KS * KS)   # 576


def build_nc(n_cores=8, H=256, B=8):
    """SPMD program for one core holding one [CIN, H, W_] image."""
    nblk = H // B                    # row blocks per core
    NR = B + 2                       # rows per block incl halo
    nchunk = (H * W_) // 512         # 2-row conv chunks per core
    TOT = float(n_cores * H * W_)    # BN count

    nc = bacc.Bacc(None, num_devices=n_cores)

    X = nc.dram_tensor("x", [CIN, H * W_], F32, kind="ExternalInput")
    M = nc.dram_tensor("mask", [CIN, H * W_], F32R, kind="ExternalInput")
    WTB = nc.dram_tensor("wtb", [128, 6 * COUT], BF16, kind="ExternalInput")
    BPB = nc.dram_tensor("bpb", [1, COUT], BF16, kind="ExternalInput")
    ONES2 = nc.dram_tensor("ones2", [128, 2], F32R, kind="ExternalInput")
    T3V = nc.dram_tensor("t3v", [NR, B], F32R, kind="ExternalInput")
    GAM = nc.dram_tensor("gam", [COUT, 1], F32, kind="ExternalInput")
    BET = nc.dram_tensor("bet", [COUT, 1], F32, kind="ExternalInput")

    OUT = nc.dram_tensor("out", [COUT, H * W_], F32, kind="ExternalOutput")
    UPD = nc.dram_tensor("upd", [H, W_], F32, kind="ExternalOutput")

    cc_in = nc.dram_tensor("ccin", [COUT, 2], F32)
    cc_out = nc.dram_tensor("ccout", [COUT, 2], F32,
                            addr_space="Shared" if n_cores > 4 else "Local")

    with tile.TileContext(nc) as tc, ExitStack() as ctx:
        nc.gpsimd.load_library(library_config.mlp)

        const = ctx.enter_context(tc.tile_pool(name="const", bufs=1))
        iox = ctx.enter_context(tc.tile_pool(name="iox", bufs=1))
        iom = ctx.enter_context(tc.tile_pool(name="iom", bufs=1))
        xmp = ctx.enter_context(tc.tile_pool(name="xmp", bufs=1))
        sev = ctx.enter_context(tc.tile_pool(name="sev", bufs=1))
        usb = ctx.enter_context(tc.tile_pool(name="usb", bufs=2))
        upd1 = ctx.enter_context(tc.tile_pool(name="upd1", bufs=1))
        upd2 = ctx.enter_context(tc.tile_pool(name="upd2", bufs=2))
        bcp = ctx.enter_context(tc.tile_pool(name="bcp", bufs=2))
        sqp = ctx.enter_context(tc.tile_pool(name="sqp", bufs=1))
        otp = ctx.enter_context(tc.tile_pool(name="otp", bufs=3))
        psc = ctx.enter_context(tc.tile_pool(name="psc", bufs=5, space="PSUM"))
        pss = ctx.enter_context(tc.tile_pool(name="pss", bufs=2, space="PSUM"))
        psv = ctx.enter_context(tc.tile_pool(name="psv", bufs=1, space="PSUM"))

        # ---- constants ----
        wt_b = const.tile([128, 6 * COUT], BF16)
        nc.sync.dma_start(wt_b[:], WTB[:])
        bp_b = const.tile([1, COUT], BF16)
        nc.sync.dma_start(bp_b[:], BPB[:])
        ones2_t = const.tile([128, 2], F32R)
        nc.sync.dma_start(ones2_t[:], ONES2[:])
        t3v_t = const.tile([NR, B], F32R)
        nc.sync.dma_start(t3v_t[:], T3V[:])
        gam_t = const.tile([COUT, 1], F32)
        nc.sync.dma_start(gam_t[:], GAM[:])
        bet_t = const.tile([COUT, 1], F32)
        nc.sync.dma_start(bet_t[:], BET[:])
        eps_t = const.tile([COUT, 1], F32)
        nc.vector.memset(eps_t[:], EPS_BN)
        sum_slots = const.tile([COUT, nchunk], F32)
        sq_slots = const.tile([COUT, nchunk // 2], F32)
        prebn = const.tile([COUT, H * W_], BF16)

        # two persistent padded xm buffers; guard cols zeroed once.
        # T0 (parts 0:64): col c = img col c-1 (writes 1:257); col 0 guard.
        # T1 (parts 64:128): col c = img col c (writes 0:256); col 256 guard.
        xm_tiles = []
        for i in range(2):
            t = const.tile([128, NR * 258], BF16, tag=f"xm{i}")
            nc.vector.memset(t[:, :].bitcast(F32), 0.0)
            xm_tiles.append(t)
        # s_rows tiles with zero guard cols 0 and 257
        sr_tiles = []
        for i in range(2):
            t = const.tile([NR, 258], F32R, tag=f"sr{i}")
            nc.vector.memset(t[:, :].bitcast(F32), 0.0)
            sr_tiles.append(t)
        # v (row 0) / mru (row 1) strips per block, bf16
        vs_tile = const.tile([1, B * W_], BF16)
        ms_tile = const.tile([1, B * W_], BF16)

        # ---- software-pipelined block loop: iteration k runs the mask/
        # update pipeline for block k+1 interleaved with convs of block k.
        blk_state = {}

        def mask_load(k):
            r0 = k * B
            first, last = (k == 0), (k == nblk - 1)
            x_t = iox.tile([CIN, NR * W_], F32, tag="x_t")
            m_t = iom.tile([128, NR * W_], F32R, tag="m_t")
            lo = max(r0 - 1, 0)
            hi = min(r0 + B + 1, H)
            dst0 = (lo - (r0 - 1)) * W_
            nr = hi - lo
            nc.sync.dma_start(
                x_t[:, dst0:dst0 + nr * W_],
                bass.AP(tensor=X, offset=lo * W_, ap=[[H * W_, CIN], [1, nr * W_]]))
            nc.sync.dma_start(
                m_t[0:CIN, dst0:dst0 + nr * W_],
                bass.AP(tensor=M, offset=lo * W_, ap=[[H * W_, CIN], [1, nr * W_]]))
            if first:
                nc.vector.memset(x_t[:, 0:W_], 0.0)
                nc.vector.memset(m_t[0:CIN, 0:W_].bitcast(F32), 0.0)
            if last:
                nc.vector.memset(x_t[:, (NR - 1) * W_:], 0.0)
                nc.vector.memset(m_t[0:CIN, (NR - 1) * W_:].bitcast(F32), 0.0)
            # mask rows shifted +1 into upper partitions (SBUF->SBUF DMA)
            nc.sync.dma_start(m_t[64:128, 0:(NR - 1) * W_],
                              m_t[0:CIN, W_:NR * W_])
            blk_state[k] = (x_t, m_t)

        def mask_strips(k):
            x_t, m_t = blk_state[k]
            m3 = m_t[:, :].rearrange("p (r c) -> p r c", c=W_)
            # strip q: rhs rows {4q, 4q+2}; out [2,512] =
            #   [s(4q), s(4q+2); s(4q+1), s(4q+3)]   (row indices in block)
            s_evac = sev.tile([2, NR * W_ // 2], F32R, tag="s_evac")
            nq = NR // 4
            for q in range(nq):
                ps_s = pss.tile([2, 512], F32, tag="ps_s")
                nc.tensor.matmul(ps_s[:], ones2_t[:],
                                 m3[:, 4 * q:4 * q + 4:2, :],
                                 start=True, stop=True)
                nc.scalar.copy(s_evac[:, q * 512:(q + 1) * 512], ps_s[:])
            ps_s = pss.tile([2, 512], F32, tag="ps_s")
            nc.tensor.matmul(ps_s[:, 0:256], ones2_t[:], m3[:, NR - 2, :],
                             start=True, stop=True)
            nc.scalar.copy(s_evac[:, nq * 512:nq * 512 + 256], ps_s[:, 0:256])
            # relayout: s_evac row h holds rows (4q+2w+h) in order
            s_rows = sr_tiles[k % 2]
            nc.sync.dma_start(s_rows[0:NR - 1:2, 1:257], s_evac[0:1, :])
            nc.sync.dma_start(s_rows[1:NR:2, 1:257], s_evac[1:2, :])

        def xm_build(k):
            x_t, m_t = blk_state[k]
            m3 = m_t[:, :].rearrange("p (r c) -> p r c", c=W_)
            xm = xm_tiles[k % 2]
            xm3 = xm[:, :].rearrange("p (r c) -> p r c", c=258)
            x3 = x_t[:, :].rearrange("p (r c) -> p r c", c=W_)
            nc.vector.tensor_tensor(
                xm3[0:64, :, 1:257], x3[:, :, :], m3[0:CIN, :, :], op=ALU.mult)
            nc.vector.tensor_tensor(
                xm3[64:128, :, 0:256], x3[:, :, :], m3[0:CIN, :, :], op=ALU.mult)

        def mask_update(k):
            r0 = k * B
            s_rows = sr_tiles[k % 2]
            ps_V = psv.tile([B, W_], F32, tag="ps_V")
            for w in range(3):
                nc.tensor.matmul(ps_V[:], t3v_t[:], s_rows[:, w:w + 256],
                                 start=(w == 0), stop=(w == 2))
            u_sb = usb.tile([B, W_], F32, tag="u_sb")
            nc.scalar.copy(u_sb[:], ps_V[:])

            vh = u_sb
            u_clip = upd2.tile([B, W_], F32, tag="u_clip")
            nc.vector.tensor_scalar_min(u_clip[:], vh[:], 1.0)
            upde = upd1.tile([B, W_], F32, tag="upde")
            nc.vector.tensor_scalar_add(upde[:], vh[:], EPS_MASK)
            rec = upd1.tile([B, W_], F32, tag="rec")
            nc.vector.reciprocal(rec[:], upde[:])
            mru_r = upd2.tile([B, W_], BF16, tag="mru_r")
            nc.vector.scalar_tensor_tensor(
                out=mru_r[:], in0=rec[:], scalar=SLIDE, in1=u_clip[:],
                op0=ALU.mult, op1=ALU.mult)
            v_r = upd2.tile([B, W_], BF16, tag="v_r")
            nc.vector.scalar_tensor_tensor(
                out=v_r[:], in0=upde[:], scalar=1.0, in1=u_clip[:],
                op0=ALU.mult, op1=ALU.mult)

            nc.sync.dma_start(
                bass.AP(tensor=UPD, offset=r0 * W_, ap=[[W_, B], [1, W_]]),
                u_clip[:])
            nc.sync.dma_start(vs_tile[:], v_r[:])
            nc.sync.dma_start(ms_tile[:], mru_r[:])

        def conv_chunks(k):
            r0 = k * B
            xm = xm_tiles[k % 2]
            xm3 = xm[:, :].rearrange("p (r c) -> p r c", c=258)
            vs = vs_tile
            ms = ms_tile
            bcs = []
            pscs = []
            for j in range(4):
                mru_bc = bcp.tile([128, 512], BF16, tag="mru_bc")
                nc.gpsimd.partition_broadcast(
                    mru_bc[:], ms[0:1, j * 512:(j + 1) * 512])
                bcs.append(mru_bc)
                ps_c = psc.tile([COUT, 512], F32, tag="ps_c")
                pscs.append(ps_c)
            for j in range(4):
                for ky in range(3):
                    nc.tensor.matmul(
                        pscs[j][:],
                        wt_b[:, ky * COUT:(ky + 1) * COUT],
                        xm3[:, 2 * j + ky:2 * j + ky + 2, 0:256],
                        start=(ky == 0), stop=False)
            for j in range(4):
                for ky in range(3):
                    nc.tensor.matmul(
                        pscs[j][:],
                        wt_b[64:128, (3 + ky) * COUT:(4 + ky) * COUT],
                        xm3[64:128, 2 * j + ky:2 * j + ky + 2, 1:257],
                        start=False, stop=False)
            for j in range(4):
                nc.tensor.matmul(pscs[j][:], bp_b[:],
                                 vs[0:1, j * 512:(j + 1) * 512],
                                 start=False, stop=True)
            for j in range(4):
                ci = 4 * k + j
                col = (r0 + 2 * j) * W_
                nc.vector.scalar_tensor_tensor(
                    out=prebn[:, col:col + 512], in0=pscs[j][:], scalar=0.0,
                    in1=bcs[j][:], op0=ALU.add, op1=ALU.mult,
                    accum_out=sum_slots[:, ci:ci + 1])
            for h in range(2):
                col = (r0 + 2 * h * 2) * W_
                sq_scr = sqp.tile([COUT, 1024], BF16, tag="sq_scr")
                nc.scalar.activation(
                    sq_scr[:], prebn[:, col:col + 1024], ACTF.Square,
                    accum_out=sq_slots[:, 2 * k + h:2 * k + h + 1])

        # prologue: full mask pipeline for block 0
        mask_load(0)
        mask_strips(0)
        mask_update(0)
        xm_build(0)
        for k in range(nblk):
            if k + 1 < nblk:
                mask_load(k + 1)
            conv_chunks(k)
            if k + 1 < nblk:
                mask_strips(k + 1)
                xm_build(k + 1)
                mask_update(k + 1)
            blk_state.pop(k, None)

        # ---- BN stats: reduce, all-reduce, affine coeffs ----
        cc_sb = const.tile([COUT, 2], F32)
        nc.vector.tensor_reduce(cc_sb[:, 0:1], sum_slots[:],
                                axis=mybir.AxisListType.X, op=ALU.add)
        nc.vector.tensor_reduce(cc_sb[:, 1:2], sq_slots[:],
                                axis=mybir.AxisListType.X, op=ALU.add)
        nc.sync.dma_start(cc_in[:], cc_sb[:])
        nc.gpsimd.collective_compute(
            "AllReduce", ALU.add,
            replica_groups=[list(range(n_cores))],
            ins=[cc_in.ap().opt()], outs=[cc_out.ap().opt()])
        st_sb = const.tile([COUT, 2], F32)
        nc.sync.dma_start(st_sb[:], cc_out[:])
        mean_t = const.tile([COUT, 1], F32)
        nc.vector.tensor_scalar_mul(mean_t[:], st_sb[:, 0:1], 1.0 / TOT)
        e2_t = const.tile([COUT, 1], F32)
        nc.vector.tensor_scalar_mul(e2_t[:], st_sb[:, 1:2], 1.0 / TOT)
        msq_t = const.tile([COUT, 1], F32)
        nc.vector.tensor_mul(msq_t[:], mean_t[:], mean_t[:])
        var_t = const.tile([COUT, 1], F32)
        nc.vector.tensor_sub(var_t[:], e2_t[:], msq_t[:])
        std_t = const.tile([COUT, 1], F32)
        nc.scalar.activation(std_t[:], var_t[:], ACTF.Sqrt, bias=eps_t[:])
        rstd_t = const.tile([COUT, 1], F32)
        nc.vector.reciprocal(rstd_t[:], std_t[:])
        scale_t = const.tile([COUT, 1], F32)
        nc.vector.tensor_mul(scale_t[:], gam_t[:], rstd_t[:])
        tmp_t = const.tile([COUT, 1], F32)
        nc.vector.tensor_mul(tmp_t[:], mean_t[:], scale_t[:])
        bias_t = const.tile([COUT, 1], F32)
        nc.vector.tensor_sub(bias_t[:], bet_t[:], tmp_t[:])

        # ---- pass 2: out = relu(scale*prebn + bias) ----
        P2 = 1024
        for i in range(0, H * W_, P2):
            o_t = otp.tile([COUT, P2], F32, tag="o_t")
            nc.scalar.activation(o_t[:], prebn[:, i:i + P2], ACTF.Relu,
                                 bias=bias_t[:], scale=scale_t[:])
            nc.sync.dma_start(OUT[:, i:i + P2], o_t[:])

    return nc


def make_host_inputs(x_i, mask_i, W, b, gamma, beta, B=8):
    """Per-core in_map for one image shard (host-side constant prep)."""
    NR = B + 2
    WTb = np.zeros((128, 6 * COUT), np.float32)
    for ky in range(KS):
        # fused pair (ky, kx=0) lower / (ky, kx=1) upper
        WTb[0:64, ky * COUT:(ky + 1) * COUT] = W[:, :, ky, 0].T
        WTb[64:128, ky * COUT:(ky + 1) * COUT] = W[:, :, ky, 1].T
        # single (ky, kx=2) upper
        WTb[64:128, (3 + ky) * COUT:(4 + ky) * COUT] = W[:, :, ky, 2].T
    ones2 = np.zeros((128, 2), np.float32)
    ones2[0:64, 0] = 1.0
    ones2[64:128, 1] = 1.0
    T3V = np.zeros((NR, B), np.float32)
    for j in range(B):
        T3V[j:j + 3, j] = 1.0
    return {
        "x": np.ascontiguousarray(x_i, dtype=np.float32).reshape(CIN, -1),
        "mask": np.ascontiguousarray(mask_i, dtype=np.float32).reshape(CIN, -1),
        "wtb": WTb.astype(ml_dtypes.bfloat16),
        "bpb": (b / SLIDE).reshape(1, COUT).astype(ml_dtypes.bfloat16),
        "ones2": ones2,
        "t3v": T3V,
        "gam": gamma.reshape(COUT, 1).astype(np.float32),
        "bet": beta.reshape(COUT, 1).astype(np.float32),
    }


_NC_CACHE = {}


def kernel(x, mask, W, b, gamma, beta):
    x = np.asarray(x)
    mask = np.asarray(mask)
    W = np.asarray(W)
    b = np.asarray(b)
    gamma = np.asarray(gamma)
    beta = np.asarray(beta)
    N, _, H, _ = x.shape
    n_cores = N
    key = (n_cores, H)
    if key not in _NC_CACHE:
        nc = build_nc(n_cores=n_cores, H=H)
        nc.finalize()
        _NC_CACHE[key] = nc
    nc = _NC_CACHE[key]

    in_maps = [make_host_inputs(x[i], mask[i], W, b, gamma, beta)
               for i in range(n_cores)]
    res = run_bass_kernel_spmd(nc, in_maps, core_ids=list(range(n_cores)),
                               trace=bool(os.environ.get("KERNEL_TRACE")))
    out = np.stack([res.results[i]["out"].reshape(COUT, H, W_)
                    for i in range(n_cores)])
    upd = np.stack([res.results[i]["upd"] for i in range(n_cores)])
    update_full = np.broadcast_to(upd[:, None, :, :], (N, COUT, H, W_))
    kernel.last_result = res
    return out, update_full
